# revision 14
# baseline (speedup 1.0000x reference)
"""Trainium2 Bass kernel for nn_LocalState_1580547972191 (sparse_attention).

Contract: kernel(**inputs) takes FULL unsharded inputs (as from setup_inputs()),
returns FULL output [4, 512, 2048] f32. Internally shards across 8 NeuronCores:
core = (b, hg) with b = batch, hg = head-group (heads {2hg, 2hg+1}).

Algorithm (per core), validated against the reference in fp64/fp32:
- The decay bias -g(s)|t-s| with g(s) >= ~0.28 makes attention effectively
  banded: weights for |t-s| > 256 are < e^-70 relative -> exactly 0 in fp32.
  Each 128-query block attends to a 640-wide key window (128-aligned).
- Freq bias cos(2*pi*(t-s)/p) = cos_p(t)cos_p(s) + sin_p(t)sin_p(s) is rank-2:
  folded into the QK^T matmul via 8 augmented rows.
- Decay bias applied as one fused DVE op: S2 = (D * (-g_p)) + S where D is a
  host-precomputed |t-s| pattern (5 distinct patterns) with the diagonal
  entries set to 1e4 (folds the eye-mask: exp(-1e4*g) = 0 = exp(-100)/sigma).
- No-max softmax: logits bounded (~15), so exp without max subtraction is
  safe in f32; sigma accumulated by the Exp activation's accum_out.
- PV needs W[t,s]; W'[s,t] tiles are transposed via the DMA xbar engines.
- time_sig recovered from 8 augmented content rows (cos/sin) post-PV; the
  cos(s)*cc + sin(s)*ss pair-sum is folded into the proj matmul by
  duplicating the tsig columns of W_proj.
- proj partial computed on-core; host sums the two head-group partials.
  Residual x, b_proj, and W_proj@b_content folded in on the hg=0 core.
"""
import math
import sys

sys.path.insert(0, "/opt/trn_rl_repo")

import ml_dtypes
import numpy as np

HEADS, NF, ND = 4, 4, 4
B, C, T = 4, 512, 2048
NBLK, WIN = 16, 512
DIAG_BIG = 1.0e4
BF16 = ml_dtypes.bfloat16

_CACHE = {}


def _w0_of_block(i):
    return 128 * min(max(i - 1, 0), 12)


def _tt_first_block(tt):
    for i in range(NBLK):
        base = min(max(i - 2, 0), 11)
        if base <= tt <= base + 4:
            return i
    raise AssertionError


def _TL(pool, shape, dtype, tag):
    return pool.tile(shape, dtype, name=tag, tag=tag)


def _build_nc():
    import concourse.mybir as mybir
    import concourse.tile as tile
    from concourse import bacc

    dt = mybir.dt
    f32, bf16 = dt.float32, dt.bfloat16
    Alu = mybir.AluOpType
    Act = mybir.ActivationFunctionType

    nc = bacc.Bacc("TRN2", target_bir_lowering=False, debug=False, num_devices=8)

    # ---- DRAM I/O (per-core shards, host-prepared) ----
    xf_d = nc.dram_tensor("xf", [C, T], f32, kind="ExternalInput")
    xb_d = nc.dram_tensor("xb", [C, T], bf16, kind="ExternalInput")
    wqkc_d = nc.dram_tensor("wqkc", [C, 792], bf16, kind="ExternalInput")
    smalls_d = nc.dram_tensor("smalls", [128, 16], f32, kind="ExternalInput")
    cs_d = nc.dram_tensor("cs", [8, T], bf16, kind="ExternalInput")
    csT_d = nc.dram_tensor("csT", [T, 8], bf16, kind="ExternalInput")
    d5_d = nc.dram_tensor("d5", [128, 4, WIN], f32, kind="ExternalInput")
    wp12_d = nc.dram_tensor("wp12", [256, C], bf16, kind="ExternalInput")
    wp3_d = nc.dram_tensor("wp3d", [16, C], bf16, kind="ExternalInput")
    out_d = nc.dram_tensor("out", [C, T], f32, kind="ExternalOutput")

    with tile.TileContext(nc) as tc:
        sing = tc.alloc_tile_pool(name="sing", bufs=1)
        work = tc.alloc_tile_pool(name="work", bufs=3)
        outp = tc.alloc_tile_pool(name="outp", bufs=2)
        xfc = tc.alloc_tile_pool(name="xfc", bufs=3)
        ps_s = tc.alloc_tile_pool(name="ps_s", bufs=3, space="PSUM")
        ps_pv = tc.alloc_tile_pool(name="ps_pv", bufs=3, space="PSUM")
        ps_sm = tc.alloc_tile_pool(name="ps_sm", bufs=2, space="PSUM")

        # ---- load persistent inputs ----
        xb = [_TL(sing, [128, T], bf16, tag=f"xb{k}") for k in range(4)]
        for k in range(4):
            nc.sync.dma_start(out=xb[k], in_=xb_d[128 * k:128 * (k + 1), :])
        wqkc = [_TL(sing, [128, 792], bf16, tag=f"wqkc{k}") for k in range(4)]
        for k in range(4):
            nc.sync.dma_start(out=wqkc[k], in_=wqkc_d[128 * k:128 * (k + 1), :])
        wqT = [w[:, 0:256] for w in wqkc]
        wkT = [w[:, 256:512] for w in wqkc]
        wcT = [w[:, 512:768] for w in wqkc]
        wfdT = [w[:, 768:792] for w in wqkc]
        smalls = _TL(sing, [128, 16], f32, tag="smalls")
        nc.sync.dma_start(out=smalls, in_=smalls_d[:, :])
        bq_sb = smalls[:, 0:2]
        bk_sb = smalls[:, 2:4]
        beff_sb = smalls[:, 4:8]
        bf_sb = [smalls[0:8, 8:9], smalls[0:8, 9:10]]
        bqd_sb = smalls[0:8, 10:11]
        gco_sb = smalls[0:8, 11:13]
        cs_sb = _TL(sing, [8, T], bf16, tag="cs")
        nc.sync.dma_start(out=cs_sb, in_=cs_d[:, :])
        d5_sb = _TL(sing, [128, 4, WIN], f32, tag="d5")
        nc.sync.dma_start(out=d5_sb, in_=d5_d[:, :, :])
        wp12_sb = [_TL(sing, [128, C], bf16, tag=f"wp{k}") for k in range(2)]
        for k in range(2):
            nc.sync.dma_start(out=wp12_sb[k], in_=wp12_d[128 * k:128 * (k + 1), :])
        wp3_sb = [_TL(sing, [8, C], bf16, tag=f"wp3_{h}") for h in range(2)]
        for h in range(2):
            nc.sync.dma_start(out=wp3_sb[h], in_=wp3_d[8 * h:8 * (h + 1), :])

        # contentT aux columns (cos/sin by t) straight from DRAM
        CT_sb = _TL(sing, [128, NBLK, 264], bf16, tag="CT")
        nc.sync.dma_start(
            out=CT_sb[:, :, 256:264],
            in_=csT_d.ap().rearrange("(tt p) c -> p tt c", p=128),
        )

        # ---- interleaved projection chunks + attention block groups ----
        Q_sb = [_TL(sing, [128, T], bf16, tag=f"Q{h}") for h in range(2)]
        K_sb = [_TL(sing, [128, T], bf16, tag=f"K{h}") for h in range(2)]
        fqh = [_TL(sing, [8, T], bf16, tag=f"fqh{h}") for h in range(2)]
        qd_sb = _TL(sing, [8, T], f32, tag="qd")
        sig_sb = _TL(sing, [8, T], f32, tag="sig")
        gneg_sb = _TL(sing, [128, 2 * NBLK], f32, tag="gneg")
        Qaux = [_TL(sing, [8, T], bf16, tag=f"Qaux{h}") for h in range(2)]
        Res = [_TL(sing, [128, T], bf16, tag=f"Res{h}") for h in range(2)]
        Tsaux = [_TL(sing, [8, T], bf16, tag=f"Tsaux{h}") for h in range(2)]
        prod8 = [_TL(sing, [8, T], bf16, tag=f"prod8{h}") for h in range(2)]
        WnT = _TL(sing, [128, NBLK, 2, WIN], bf16, tag="WnT")
        pat_of = lambda i: {0: 0, 14: 2, 15: 3}.get(i, 1)

        def emit_proj_chunk(n):
            cols = slice(512 * n, 512 * (n + 1))
            for h in range(2):
                pq = _TL(ps_pv, [128, 512], f32, tag="proj")
                for k in range(4):
                    nc.tensor.matmul(pq, wqT[k][:, 128 * h:128 * (h + 1)],
                                     xb[k][:, cols], start=(k == 0), stop=(k == 3))
                nc.scalar.activation(out=Q_sb[h][:, cols], in_=pq,
                                     func=Act.Identity, bias=bq_sb[:, h:h + 1])
                pk = _TL(ps_pv, [128, 512], f32, tag="proj")
                for k in range(4):
                    nc.tensor.matmul(pk, wkT[k][:, 128 * h:128 * (h + 1)],
                                     xb[k][:, cols], start=(k == 0), stop=(k == 3))
                nc.scalar.activation(out=K_sb[h][:, cols], in_=pk,
                                     func=Act.Identity, bias=bk_sb[:, h:h + 1])
                pf = _TL(ps_sm, [8, 512], f32, tag="sm")
                for k in range(4):
                    nc.tensor.matmul(pf, wfdT[k][:, 8 * h:8 * h + 8],
                                     xb[k][:, cols], start=(k == 0), stop=(k == 3))
                nc.scalar.activation(out=fqh[h][:, cols], in_=pf,
                                     func=Act.Identity, bias=bf_sb[h][:, 0:1])
            pd = _TL(ps_sm, [8, 512], f32, tag="sm")
            for k in range(4):
                nc.tensor.matmul(pd, wfdT[k][:, 16:24],
                                 xb[k][:, cols], start=(k == 0), stop=(k == 3))
            nc.scalar.activation(out=qd_sb[:, cols], in_=pd,
                                 func=Act.Identity, bias=bqd_sb[:, 0:1])
            for tt in range(4 * n, 4 * n + 4):   # contentT tiles
                rows = slice(128 * tt, 128 * (tt + 1))
                pc = _TL(ps_pv, [128, 256], f32, tag="proj")
                for k in range(4):
                    nc.tensor.matmul(pc, xb[k][:, rows], wcT[k],
                                     start=(k == 0), stop=(k == 3))
                nc.scalar.activation(out=CT_sb[:, tt, 0:256], in_=pc, func=Act.Copy)
            nc.scalar.activation(out=sig_sb[:, cols], in_=qd_sb[:, cols],
                                 func=Act.Sigmoid)
            for i in range(4 * n, 4 * n + 4):
                pg = _TL(ps_sm, [128, 2], f32, tag="sm")
                nc.tensor.matmul(pg, sig_sb[:, 128 * i:128 * (i + 1)], gco_sb,
                                 start=True, stop=True)
                nc.vector.tensor_copy(out=gneg_sb[:, 2 * i:2 * i + 2], in_=pg)
            for h in range(2):
                nc.gpsimd.tensor_mul(Qaux[h][:, cols], cs_sb[:, cols],
                                     fqh[h][:, cols])

        def emit_block_pair(i):
            s0, w0 = 128 * i, _w0_of_block(i)
            base = min(max(i - 1, 0), 12)
            wn2 = _TL(work, [128, 2 * WIN], bf16, tag="Wn2")
            for h in range(2):
                sp = _TL(ps_s, [128, WIN], f32, tag="S")
                nc.tensor.matmul(sp, Q_sb[h][:, s0:s0 + 128],
                                 K_sb[h][:, w0:w0 + WIN], start=True, stop=False)
                nc.tensor.matmul(sp, Qaux[h][:, s0:s0 + 128],
                                 cs_sb[:, w0:w0 + WIN], start=False, stop=True)
                s2 = _TL(work, [128, WIN], f32, tag="S2")
                nc.vector.scalar_tensor_tensor(
                    out=s2, in0=d5_sb[:, pat_of(i), :],
                    scalar=gneg_sb[:, 2 * i + h:2 * i + h + 1],
                    in1=sp, op0=Alu.mult, op1=Alu.add)
                wexp = _TL(work, [128, WIN], bf16, tag="Wexp")
                sigma = _TL(work, [128, 1], f32, tag="sigma")
                nc.scalar.activation(out=wexp, in_=s2, func=Act.Exp, accum_out=sigma)
                recip = _TL(work, [128, 1], f32, tag="recip")
                nc.vector.reciprocal(out=recip, in_=sigma)
                nc.vector.tensor_scalar_mul(wn2[:, WIN * h:WIN * (h + 1)], wexp, recip)
            nc.scalar.dma_start_transpose(
                out=WnT[:, i, :, :].rearrange("p hh (k e) -> p (hh k) e", e=128),
                in_=wn2)
            for h in range(2):
                om = _TL(ps_pv, [128, 128], f32, tag="proj")
                oa = _TL(ps_sm, [8, 128], f32, tag="sm")
                for j in range(4):
                    tt = base + j
                    rhs = WnT[:, i, h, 128 * j:128 * (j + 1)]
                    nc.tensor.matmul(om, CT_sb[:, tt, 128 * h:128 * (h + 1)],
                                     rhs, start=(j == 0), stop=(j == 3))
                    nc.tensor.matmul(oa, CT_sb[:, tt, 256:264],
                                     rhs, start=(j == 0), stop=(j == 3))
                nc.vector.tensor_copy(out=Res[h][:, s0:s0 + 128], in_=om)
                nc.scalar.activation(out=Tsaux[h][:, s0:s0 + 128], in_=oa,
                                     func=Act.Copy)

        def emit_out_chunk(n):
            cols = slice(512 * n, 512 * (n + 1))
            for h in range(2):
                nc.gpsimd.tensor_mul(prod8[h][:, cols], cs_sb[:, cols],
                                     Tsaux[h][:, cols])
            for ot in range(4):
                osl = slice(128 * ot, 128 * (ot + 1))
                xft = _TL(xfc, [128, 512], f32, tag="xft")
                nc.sync.dma_start(out=xft, in_=xf_d[osl, cols])
                pp = _TL(ps_pv, [128, 512], f32, tag="proj")
                nc.tensor.matmul(pp, wp12_sb[0][:, osl], Res[0][:, cols],
                                 start=True, stop=False)
                nc.tensor.matmul(pp, wp12_sb[1][:, osl], Res[1][:, cols],
                                 start=False, stop=False)
                nc.tensor.matmul(pp, wp3_sb[0][:, osl], prod8[0][:, cols],
                                 start=False, stop=False)
                nc.tensor.matmul(pp, wp3_sb[1][:, osl], prod8[1][:, cols],
                                 start=False, stop=True)
                ob = _TL(outp, [128, 512], f32, tag="ob")
                if (ot + n) % 2 == 0:
                    nc.vector.scalar_tensor_tensor(
                        out=ob, in0=pp, scalar=beff_sb[:, ot:ot + 1],
                        in1=xft, op0=Alu.add, op1=Alu.add)
                else:
                    tmp = _TL(outp, [128, 512], f32, tag="tmp")
                    nc.scalar.activation(out=tmp, in_=pp, func=Act.Identity,
                                         bias=beff_sb[:, ot:ot + 1])
                    nc.gpsimd.tensor_add(ob, tmp, xft)
                nc.sync.dma_start(out=out_d[osl, cols], in_=ob)

        for n in range(4):
            emit_proj_chunk(n)
        for i in range(NBLK):
            emit_block_pair(i)
        for n in range(4):
            emit_out_chunk(n)

        for pool in (ps_sm, ps_pv, ps_s, xfc, outp, work, sing):
            pool.release()

    nc.compile()
    return nc


def _cos_sin():
    t = np.arange(T, dtype=np.float64)
    per = np.arange(1, NF + 1, dtype=np.float64)
    ang = 2 * math.pi * t[None, :] / per[:, None]
    return np.cos(ang).astype(np.float32), np.sin(ang).astype(np.float32)


def _d_patterns():
    d5 = np.empty((128, 4, WIN), np.float32)
    p = np.arange(128)[:, None]
    j = np.arange(WIN)[None, :]
    for k, off in enumerate((0, 128, 256, 384)):
        d = np.abs(p + off - j).astype(np.float32)
        d[p + off == j] = DIAG_BIG
        d5[:, k, :] = d
    return d5


_COS, _SIN = _cos_sin()
_D5 = _d_patterns()


def _prep_core_inputs(inputs, b, hg):
    f32 = np.float32
    x_b = np.ascontiguousarray(np.asarray(inputs["x"])[b], dtype=f32)
    hsl = slice(hg * 256, (hg + 1) * 256)
    fsl = slice(hg * 8, (hg + 1) * 8)
    s = f32(1.0 / math.sqrt(128.0))
    cosT, sinT = _COS, _SIN

    Wq = np.asarray(inputs["W_query"], f32)[hsl] * s
    bq = np.asarray(inputs["b_query"], f32)[hsl] * s
    Wk = np.asarray(inputs["W_key"], f32)[hsl]
    bk = np.asarray(inputs["b_key"], f32)[hsl]
    Wc = np.asarray(inputs["W_content"], f32)[hsl]
    Wf = np.asarray(inputs["W_qfreq"], f32)[fsl] * f32(0.5)
    bf = np.asarray(inputs["b_qfreq"], f32)[fsl] * f32(0.5)
    Wd = np.asarray(inputs["W_qdecay"], f32)[fsl]
    bd = np.asarray(inputs["b_qdecay"], f32)[fsl]
    Wp = np.asarray(inputs["W_proj"], f32)
    Wp_hg = Wp[:, hg * 264:(hg + 1) * 264]

    # wfdT cols: [fq_h0 x2 (dup), fq_h1 x2 (dup), qd_h0, qd_h1]
    wfd = np.concatenate([Wf[0:4], Wf[0:4], Wf[4:8], Wf[4:8],
                          Wd[0:4], Wd[4:8]], axis=0)  # [24, 512]

    gco = np.zeros((8, 2), f32)
    dvec = -(np.arange(1, ND + 1, dtype=f32) / 4)
    gco[0:4, 0] = dvec
    gco[4:8, 1] = dvec

    wp12 = np.concatenate([Wp_hg[:, 0:128].T, Wp_hg[:, 132:260].T], axis=0)
    # tsig proj cols duplicated (cos part rows 0-3, sin part rows 4-7)
    wp3d = np.concatenate([Wp_hg[:, 128:132].T, Wp_hg[:, 128:132].T,
                           Wp_hg[:, 260:264].T, Wp_hg[:, 260:264].T], axis=0)

    if hg == 0:
        b_eff = np.asarray(inputs["b_proj"], f32).copy()
        bc = np.asarray(inputs["b_content"], f32)
        for h in range(HEADS):
            b_eff += Wp[:, 132 * h:132 * h + 128] @ bc[128 * h:128 * h + 128]
        xf = x_b
    else:
        b_eff = np.zeros(C, f32)
        xf = np.zeros((C, T), f32)
    beff = np.ascontiguousarray(b_eff.reshape(4, 128).T)  # b_eff[128*ot+p] -> [p, ot]

    wqkc = np.concatenate([Wq.T, Wk.T, Wc.T, wfd.T], axis=1)  # [512, 792]
    smalls = np.zeros((128, 16), f32)
    smalls[:, 0:2] = bq.reshape(2, 128).T
    smalls[:, 2:4] = bk.reshape(2, 128).T
    smalls[:, 4:8] = beff
    smalls[0:8, 8] = np.concatenate([bf[0:4], bf[0:4]])
    smalls[0:8, 9] = np.concatenate([bf[4:8], bf[4:8]])
    smalls[0:8, 10] = np.concatenate([bd[0:4], bd[4:8]])
    smalls[0:8, 11:13] = gco
    return {
        "xf": xf,
        "xb": x_b.astype(BF16),
        "wqkc": np.ascontiguousarray(wqkc).astype(BF16),
        "smalls": smalls,
        "cs": np.concatenate([cosT, sinT], 0).astype(BF16),
        "csT": np.ascontiguousarray(np.concatenate([cosT, sinT], 0).T).astype(BF16),
        "d5": _D5,
        "wp12": np.ascontiguousarray(wp12).astype(BF16),
        "wp3d": np.ascontiguousarray(wp3d).astype(BF16),
    }


def get_nc():
    if "nc" not in _CACHE:
        _CACHE["nc"] = _build_nc()
    return _CACHE["nc"]


def make_in_maps(inputs):
    return [_prep_core_inputs(inputs, c // 2, c % 2) for c in range(8)]


def kernel(**inputs):
    from concourse.bass_utils import run_bass_kernel_spmd

    nc = get_nc()
    in_maps = make_in_maps(inputs)
    res = run_bass_kernel_spmd(nc, in_maps, core_ids=list(range(8)))
    out = np.empty((B, C, T), np.float32)
    for b in range(B):
        out[b] = res.results[2 * b]["out"] + res.results[2 * b + 1]["out"]
    return out


# revision 25
# speedup vs baseline: 1.2899x; 1.2899x over previous
"""Trainium2 Bass kernel for nn_LocalState_1580547972191 (sparse_attention).

Contract: kernel(**inputs) takes FULL unsharded inputs (as from setup_inputs()),
returns FULL output [4, 512, 2048] f32. Internally shards across 8 NeuronCores:
core = (b, hg) with b = batch, hg = head-group (heads {2hg, 2hg+1}).

Algorithm (per core), validated against the reference in fp64/fp32:
- The decay bias -g(s)|t-s| with g(s) >= ~0.28 makes attention effectively
  banded: weights for |t-s| > 256 are < e^-70 relative -> exactly 0 in fp32.
  Each 128-query block attends to a 640-wide key window (128-aligned).
- Freq bias cos(2*pi*(t-s)/p) = cos_p(t)cos_p(s) + sin_p(t)sin_p(s) is rank-2:
  folded into the QK^T matmul via 8 augmented rows.
- Decay bias applied as one fused DVE op: S2 = (D * (-g_p)) + S where D is a
  host-precomputed |t-s| pattern (5 distinct patterns) with the diagonal
  entries set to 1e4 (folds the eye-mask: exp(-1e4*g) = 0 = exp(-100)/sigma).
- No-max softmax: logits bounded (~15), so exp without max subtraction is
  safe in f32; sigma accumulated by the Exp activation's accum_out.
- PV needs W[t,s]; W'[s,t] tiles are transposed via the DMA xbar engines.
- time_sig recovered from 8 augmented content rows (cos/sin) post-PV; the
  cos(s)*cc + sin(s)*ss pair-sum is folded into the proj matmul by
  duplicating the tsig columns of W_proj.
- proj partial computed on-core; host sums the two head-group partials.
  Residual x, b_proj, and W_proj@b_content folded in on the hg=0 core.
"""
import math
import sys

sys.path.insert(0, "/opt/trn_rl_repo")

import ml_dtypes
import numpy as np

HEADS, NF, ND = 4, 4, 4
B, C, T = 4, 512, 2048
NBLK, WIN = 16, 384
DIAG_BIG = 1.0e4
BF16 = ml_dtypes.bfloat16

_CACHE = {}


def _w0_of_block(i):
    return 128 * min(max(i - 1, 0), 13)


def _tt_first_block(tt):
    for i in range(NBLK):
        base = min(max(i - 2, 0), 11)
        if base <= tt <= base + 4:
            return i
    raise AssertionError


def _TL(pool, shape, dtype, tag):
    return pool.tile(shape, dtype, name=tag, tag=tag)


def _build_nc():
    import concourse.mybir as mybir
    import concourse.tile as tile
    from concourse import bacc

    dt = mybir.dt
    f32, bf16 = dt.float32, dt.bfloat16
    Alu = mybir.AluOpType
    Act = mybir.ActivationFunctionType

    nc = bacc.Bacc("TRN2", target_bir_lowering=False, debug=False, num_devices=8)

    # ---- DRAM I/O (per-core shards, host-prepared) ----
    xb_d = nc.dram_tensor("xb", [C, T], bf16, kind="ExternalInput")
    wqkc_d = nc.dram_tensor("wqkc", [C, 792], bf16, kind="ExternalInput")
    smalls_d = nc.dram_tensor("smalls", [128, 16], f32, kind="ExternalInput")
    cs_d = nc.dram_tensor("cs", [8, T], bf16, kind="ExternalInput")
    csT_d = nc.dram_tensor("csT", [T, 8], bf16, kind="ExternalInput")
    d5_d = nc.dram_tensor("d5", [128, 3, WIN], f32, kind="ExternalInput")
    wp12_d = nc.dram_tensor("wp12", [256, C], bf16, kind="ExternalInput")
    wp3_d = nc.dram_tensor("wp3d", [16, C], bf16, kind="ExternalInput")
    out_d = nc.dram_tensor("out", [C, T], f32, kind="ExternalOutput")

    with tile.TileContext(nc) as tc:
        sing = tc.alloc_tile_pool(name="sing", bufs=1)
        work = tc.alloc_tile_pool(name="work", bufs=4)
        outp = tc.alloc_tile_pool(name="outp", bufs=2)
        ps_s = tc.alloc_tile_pool(name="ps_s", bufs=3, space="PSUM")
        ps_pv = tc.alloc_tile_pool(name="ps_pv", bufs=3, space="PSUM")
        ps_sm = tc.alloc_tile_pool(name="ps_sm", bufs=2, space="PSUM")

        # ---- load persistent inputs ----
        xb = [_TL(sing, [128, T], bf16, tag=f"xb{k}") for k in range(4)]
        wqkc = [_TL(sing, [128, 792], bf16, tag=f"wqkc{k}") for k in range(4)]
        for k in range(4):
            nc.sync.dma_start(out=wqkc[k], in_=wqkc_d[128 * k:128 * (k + 1), :])
            nc.sync.dma_start(out=xb[k], in_=xb_d[128 * k:128 * (k + 1), :])
        wqT = [w[:, 0:256] for w in wqkc]
        wkT = [w[:, 256:512] for w in wqkc]
        wcT = [w[:, 512:768] for w in wqkc]
        wfdT = [w[:, 768:792] for w in wqkc]
        smalls = _TL(sing, [128, 16], f32, tag="smalls")
        nc.sync.dma_start(out=smalls, in_=smalls_d[:, :])
        bq_sb = smalls[:, 0:2]
        bk_sb = smalls[:, 2:4]
        beff_sb = smalls[:, 4:8]
        bf_sb = [smalls[0:8, 8:9], smalls[0:8, 9:10]]
        bqd_sb = smalls[0:8, 10:11]
        gco_sb = smalls[0:8, 11:13]
        cs_sb = _TL(sing, [8, T], bf16, tag="cs")
        nc.sync.dma_start(out=cs_sb, in_=cs_d[:, :])
        d5_sb = _TL(sing, [128, 3, WIN], f32, tag="d5")
        nc.sync.dma_start(out=d5_sb, in_=d5_d[:, :, :])
        wp12_sb = [_TL(sing, [128, C], bf16, tag=f"wp{k}") for k in range(2)]
        for k in range(2):
            nc.sync.dma_start(out=wp12_sb[k], in_=wp12_d[128 * k:128 * (k + 1), :])
        wp3_sb = [_TL(sing, [8, C], bf16, tag=f"wp3_{h}") for h in range(2)]
        for h in range(2):
            nc.sync.dma_start(out=wp3_sb[h], in_=wp3_d[8 * h:8 * (h + 1), :])

        # contentT aux columns (cos/sin by t) straight from DRAM
        CT_sb = _TL(sing, [128, NBLK, 264], bf16, tag="CT")
        nc.sync.dma_start(
            out=CT_sb[:, :, 256:264],
            in_=csT_d.ap().rearrange("(tt p) c -> p tt c", p=128),
        )

        # ---- interleaved projection chunks + attention block groups ----
        Q_sb = [_TL(sing, [128, T], bf16, tag=f"Q{h}") for h in range(2)]
        K_sb = [_TL(sing, [128, T], bf16, tag=f"K{h}") for h in range(2)]
        fqh = [_TL(sing, [8, T], bf16, tag=f"fqh{h}") for h in range(2)]
        qd_sb = _TL(sing, [8, T], f32, tag="qd")
        sig_sb = _TL(sing, [8, T], f32, tag="sig")
        gneg_sb = _TL(sing, [128, 2 * NBLK], f32, tag="gneg")
        Qaux = [_TL(sing, [8, T], bf16, tag=f"Qaux{h}") for h in range(2)]
        Res = [_TL(sing, [128, T], bf16, tag=f"Res{h}") for h in range(2)]
        Tsaux = [_TL(sing, [8, T], bf16, tag=f"Tsaux{h}") for h in range(2)]
        prod8 = [_TL(sing, [8, T], bf16, tag=f"prod8{h}") for h in range(2)]
        WnT = _TL(sing, [128, NBLK, 2, WIN], bf16, tag="WnT")
        pat_of = lambda i: {0: 0, 15: 2}.get(i, 1)

        def emit_proj_chunk(n):
            cols = slice(512 * n, 512 * (n + 1))
            for h in range(2):
                pq = _TL(ps_pv, [128, 512], f32, tag="proj")
                for k in range(4):
                    nc.tensor.matmul(pq, wqT[k][:, 128 * h:128 * (h + 1)],
                                     xb[k][:, cols], start=(k == 0), stop=(k == 3))
                nc.scalar.activation(out=Q_sb[h][:, cols], in_=pq,
                                     func=Act.Identity, bias=bq_sb[:, h:h + 1])
                pk = _TL(ps_pv, [128, 512], f32, tag="proj")
                for k in range(4):
                    nc.tensor.matmul(pk, wkT[k][:, 128 * h:128 * (h + 1)],
                                     xb[k][:, cols], start=(k == 0), stop=(k == 3))
                nc.scalar.activation(out=K_sb[h][:, cols], in_=pk,
                                     func=Act.Identity, bias=bk_sb[:, h:h + 1])
                pf = _TL(ps_sm, [8, 512], f32, tag="sm")
                for k in range(4):
                    nc.tensor.matmul(pf, wfdT[k][:, 8 * h:8 * h + 8],
                                     xb[k][:, cols], start=(k == 0), stop=(k == 3))
                nc.scalar.activation(out=fqh[h][:, cols], in_=pf,
                                     func=Act.Identity, bias=bf_sb[h][:, 0:1])
            pd = _TL(ps_sm, [8, 512], f32, tag="sm")
            for k in range(4):
                nc.tensor.matmul(pd, wfdT[k][:, 16:24],
                                 xb[k][:, cols], start=(k == 0), stop=(k == 3))
            nc.scalar.activation(out=qd_sb[:, cols], in_=pd,
                                 func=Act.Identity, bias=bqd_sb[:, 0:1])
            for tt in range(4 * n, 4 * n + 4):   # contentT tiles
                rows = slice(128 * tt, 128 * (tt + 1))
                pc = _TL(ps_pv, [128, 256], f32, tag="proj")
                for k in range(4):
                    nc.tensor.matmul(pc, xb[k][:, rows], wcT[k],
                                     start=(k == 0), stop=(k == 3))
                nc.scalar.activation(out=CT_sb[:, tt, 0:256], in_=pc, func=Act.Copy)
            etmp = _TL(work, [8, 512], f32, tag="sigtmp")
            nc.scalar.activation(out=etmp, in_=qd_sb[:, cols],
                                 func=Act.Exp, scale=-1.0)
            ep1 = _TL(work, [8, 512], f32, tag="sigtmp2")
            nc.vector.tensor_scalar_add(ep1, etmp, 1.0)
            nc.vector.reciprocal(out=sig_sb[:, cols], in_=ep1)
            for i in range(4 * n, 4 * n + 4):
                pg = _TL(ps_sm, [128, 2], f32, tag="sm")
                nc.tensor.matmul(pg, sig_sb[:, 128 * i:128 * (i + 1)], gco_sb,
                                 start=True, stop=True)
                nc.vector.tensor_copy(out=gneg_sb[:, 2 * i:2 * i + 2], in_=pg)
            for h in range(2):
                nc.gpsimd.tensor_mul(Qaux[h][:, cols], cs_sb[:, cols],
                                     fqh[h][:, cols])

        def emit_block_pair(i):
            s0, w0 = 128 * i, _w0_of_block(i)
            base = min(max(i - 1, 0), 13)
            wn2 = _TL(work, [128, 2 * WIN], bf16, tag="Wn2")
            s2s, sigmas = [], []
            for h in range(2):
                sp = _TL(ps_s, [128, WIN], f32, tag="S")
                nc.tensor.matmul(sp, Q_sb[h][:, s0:s0 + 128],
                                 K_sb[h][:, w0:w0 + WIN], start=True, stop=False)
                nc.tensor.matmul(sp, Qaux[h][:, s0:s0 + 128],
                                 cs_sb[:, w0:w0 + WIN], start=False, stop=True)
                nc.vector.scalar_tensor_tensor(
                    out=sp, in0=d5_sb[:, pat_of(i), :],
                    scalar=gneg_sb[:, 2 * i + h:2 * i + h + 1],
                    in1=sp, op0=Alu.mult, op1=Alu.add)
                s2s.append(sp)
            wexps = []
            for h in range(2):
                wexp = _TL(work, [128, WIN], bf16, tag="Wexp")
                sigma = _TL(work, [128, 1], f32, tag="sigma")
                nc.scalar.activation(out=wexp, in_=s2s[h], func=Act.Exp,
                                     accum_out=sigma)
                wexps.append(wexp); sigmas.append(sigma)
            for h in range(2):
                recip = _TL(work, [128, 1], f32, tag="recip")
                nc.vector.reciprocal(out=recip, in_=sigmas[h])
                nc.vector.tensor_scalar_mul(wn2[:, WIN * h:WIN * (h + 1)],
                                            wexps[h], recip)
            nc.scalar.dma_start_transpose(
                out=WnT[:, i, :, :].rearrange("p hh (k e) -> p (hh k) e", e=128),
                in_=wn2)
            for h in range(2):
                om = _TL(ps_pv, [128, 128], f32, tag="proj")
                oa = _TL(ps_sm, [8, 128], f32, tag="sm")
                for j in range(3):
                    tt = base + j
                    rhs = WnT[:, i, h, 128 * j:128 * (j + 1)]
                    nc.tensor.matmul(om, CT_sb[:, tt, 128 * h:128 * (h + 1)],
                                     rhs, start=(j == 0), stop=(j == 2))
                    nc.tensor.matmul(oa, CT_sb[:, tt, 256:264],
                                     rhs, start=(j == 0), stop=(j == 2))
                nc.vector.tensor_copy(out=Res[h][:, s0:s0 + 128], in_=om)
                nc.scalar.activation(out=Tsaux[h][:, s0:s0 + 128], in_=oa,
                                     func=Act.Copy)

        def emit_out_ot(ot):
            osl = slice(128 * ot, 128 * (ot + 1))
            ob = _TL(outp, [128, T], f32, tag="ob")
            for n in range(4):
                cols = slice(512 * n, 512 * (n + 1))
                pp = _TL(ps_pv, [128, 512], f32, tag="proj")
                nc.tensor.matmul(pp, wp12_sb[0][:, osl], Res[0][:, cols],
                                 start=True, stop=False)
                nc.tensor.matmul(pp, wp12_sb[1][:, osl], Res[1][:, cols],
                                 start=False, stop=False)
                nc.tensor.matmul(pp, wp3_sb[0][:, osl], prod8[0][:, cols],
                                 start=False, stop=False)
                nc.tensor.matmul(pp, wp3_sb[1][:, osl], prod8[1][:, cols],
                                 start=False, stop=True)
                if n % 2 == 0:
                    nc.vector.tensor_scalar(
                        out=ob[:, cols], in0=pp, scalar1=beff_sb[:, ot:ot + 1],
                        scalar2=None, op0=Alu.add)
                else:
                    nc.scalar.activation(out=ob[:, cols], in_=pp,
                                         func=Act.Identity,
                                         bias=beff_sb[:, ot:ot + 1])
            nc.sync.dma_start(out=out_d[osl, :], in_=ob)

        for n in range(4):
            emit_proj_chunk(n)
        for i in range(NBLK):
            emit_block_pair(i)
        for n in range(4):
            cols = slice(512 * n, 512 * (n + 1))
            for h in range(2):
                nc.gpsimd.tensor_mul(prod8[h][:, cols], cs_sb[:, cols],
                                     Tsaux[h][:, cols])
        for ot in range(4):
            emit_out_ot(ot)

        for pool in (ps_sm, ps_pv, ps_s, outp, work, sing):
            pool.release()

    nc.compile()
    return nc


def _cos_sin():
    t = np.arange(T, dtype=np.float64)
    per = np.arange(1, NF + 1, dtype=np.float64)
    ang = 2 * math.pi * t[None, :] / per[:, None]
    return np.cos(ang).astype(np.float32), np.sin(ang).astype(np.float32)


def _d_patterns():
    d5 = np.empty((128, 3, WIN), np.float32)
    p = np.arange(128)[:, None]
    j = np.arange(WIN)[None, :]
    for k, off in enumerate((0, 128, 256)):
        d = np.abs(p + off - j).astype(np.float32)
        d[p + off == j] = DIAG_BIG
        d5[:, k, :] = d
    return d5


_COS, _SIN = _cos_sin()
_D5 = _d_patterns()


def _prep_core_inputs(inputs, b, hg):
    f32 = np.float32
    x_b = np.ascontiguousarray(np.asarray(inputs["x"])[b], dtype=f32)
    hsl = slice(hg * 256, (hg + 1) * 256)
    fsl = slice(hg * 8, (hg + 1) * 8)
    s = f32(1.0 / math.sqrt(128.0))
    cosT, sinT = _COS, _SIN

    Wq = np.asarray(inputs["W_query"], f32)[hsl] * s
    bq = np.asarray(inputs["b_query"], f32)[hsl] * s
    Wk = np.asarray(inputs["W_key"], f32)[hsl]
    bk = np.asarray(inputs["b_key"], f32)[hsl]
    Wc = np.asarray(inputs["W_content"], f32)[hsl]
    Wf = np.asarray(inputs["W_qfreq"], f32)[fsl] * f32(0.5)
    bf = np.asarray(inputs["b_qfreq"], f32)[fsl] * f32(0.5)
    Wd = np.asarray(inputs["W_qdecay"], f32)[fsl]
    bd = np.asarray(inputs["b_qdecay"], f32)[fsl]
    Wp = np.asarray(inputs["W_proj"], f32)
    Wp_hg = Wp[:, hg * 264:(hg + 1) * 264]

    # wfdT cols: [fq_h0 x2 (dup), fq_h1 x2 (dup), qd_h0, qd_h1]
    wfd = np.concatenate([Wf[0:4], Wf[0:4], Wf[4:8], Wf[4:8],
                          Wd[0:4], Wd[4:8]], axis=0)  # [24, 512]

    gco = np.zeros((8, 2), f32)
    dvec = -(np.arange(1, ND + 1, dtype=f32) / 4)
    gco[0:4, 0] = dvec
    gco[4:8, 1] = dvec

    wp12 = np.concatenate([Wp_hg[:, 0:128].T, Wp_hg[:, 132:260].T], axis=0)
    # tsig proj cols duplicated (cos part rows 0-3, sin part rows 4-7)
    wp3d = np.concatenate([Wp_hg[:, 128:132].T, Wp_hg[:, 128:132].T,
                           Wp_hg[:, 260:264].T, Wp_hg[:, 260:264].T], axis=0)

    if hg == 0:
        b_eff = np.asarray(inputs["b_proj"], f32).copy()
        bc = np.asarray(inputs["b_content"], f32)
        for h in range(HEADS):
            b_eff += Wp[:, 132 * h:132 * h + 128] @ bc[128 * h:128 * h + 128]
    else:
        b_eff = np.zeros(C, f32)
    beff = np.ascontiguousarray(b_eff.reshape(4, 128).T)  # b_eff[128*ot+p] -> [p, ot]

    wqkc = np.concatenate([Wq.T, Wk.T, Wc.T, wfd.T], axis=1)  # [512, 792]
    smalls = np.zeros((128, 16), f32)
    smalls[:, 0:2] = bq.reshape(2, 128).T
    smalls[:, 2:4] = bk.reshape(2, 128).T
    smalls[:, 4:8] = beff
    smalls[0:8, 8] = np.concatenate([bf[0:4], bf[0:4]])
    smalls[0:8, 9] = np.concatenate([bf[4:8], bf[4:8]])
    smalls[0:8, 10] = np.concatenate([bd[0:4], bd[4:8]])
    smalls[0:8, 11:13] = gco
    return {
        "xb": x_b.astype(BF16),
        "wqkc": np.ascontiguousarray(wqkc).astype(BF16),
        "smalls": smalls,
        "cs": np.concatenate([cosT, sinT], 0).astype(BF16),
        "csT": np.ascontiguousarray(np.concatenate([cosT, sinT], 0).T).astype(BF16),
        "d5": _D5,
        "wp12": np.ascontiguousarray(wp12).astype(BF16),
        "wp3d": np.ascontiguousarray(wp3d).astype(BF16),
    }


def get_nc():
    if "nc" not in _CACHE:
        _CACHE["nc"] = _build_nc()
    return _CACHE["nc"]


def make_in_maps(inputs):
    return [_prep_core_inputs(inputs, c // 2, c % 2) for c in range(8)]


def kernel(**inputs):
    from concourse.bass_utils import run_bass_kernel_spmd

    nc = get_nc()
    in_maps = make_in_maps(inputs)
    res = run_bass_kernel_spmd(nc, in_maps, core_ids=list(range(8)))
    x = np.asarray(inputs["x"], np.float32)
    out = np.empty((B, C, T), np.float32)
    for b in range(B):
        np.add(res.results[2 * b]["out"], res.results[2 * b + 1]["out"], out=out[b])
        out[b] += x[b]
    return out
